# revision 6
# baseline (speedup 1.0000x reference)
"""Trainium2 Bass kernel for nn_DecoderGRU (attention GRU decoder + vocab head).

Strategy (8 NeuronCores, data-parallel over batch, 8 rows/core):
  - All-fp16 tensors (weights, activations, history): halves DMA traffic and
    doubles DVE throughput where 2x modes apply; PSUM accumulation is fp32.
  - Startup: precompute inputs (featsT/attn_We/embT/W_iheT) DMA'd first,
    recurrence weights next, fcW last so the recurrence starts ~20us in.
  - Hoisted out of the 32-step loop: feat_proj, xgx (= emb @ W_ih[:, :E].T
    + b_ih + b_hh), and the fc head.
  - fc head: stationary = 128 finished h columns; the first half (steps
    0..15) is interleaved into steps 16..31 on the otherwise idle PE, with
    logits DMA'd straight from PSUM to DRAM; only the second half runs as a
    tail after the loop.
  - Per step, the serial chain is minimized: hp before gh on PE; gate
    pre-adds emitted after the energy chain; exb/recb PSUM->SBUF copies on
    ACT (frees DVE); sigmoid via raw tanh(x/2) algebra with
    scalar_tensor_tensor fusions; h_new written directly into the fp16
    history slab used by both the next step and the fc head.
"""

import threading

import numpy as np

B, R, E, H, V, L = 64, 49, 512, 512, 10000, 33
T = L - 1            # 32 decode steps
NCORES = 8
BL = B // NCORES     # 8 batch rows per core
KT = E // 128        # 4 k-tiles of 128 for E=H=512
M3H = (3 * H) // 128  # 12 m-tiles for gate dim
RSPLIT = ((0, 25), (25, 49))  # r-halves for the energy pipeline
NCH = (V + 511) // 512        # 20 vocab chunks, last = 272

_BUILD_LOCK = threading.Lock()
_BUILT = {}


def _build(has_fcb=True):
    import concourse.mybir as mybir
    import concourse.tile as tile
    from concourse import bacc

    F32 = mybir.dt.float32
    F16 = mybir.dt.float16
    AF = mybir.ActivationFunctionType
    OP = mybir.AluOpType

    nc = bacc.Bacc("TRN2", target_bir_lowering=False, debug=False,
                   num_devices=NCORES)

    # ---- DRAM I/O (everything fp16 except f32 biases and the output) ----
    featsT_d = nc.dram_tensor("featsT", [E, R, BL], F16, kind="ExternalInput")
    featsb_d = nc.dram_tensor("featsb", [E, BL, R], F16, kind="ExternalInput")
    embT_d = nc.dram_tensor("embT", [E, T * BL], F16, kind="ExternalInput")
    attn_We_d = nc.dram_tensor("attn_We", [E, H], F16, kind="ExternalInput")
    attn_Wh_d = nc.dram_tensor("attn_Wh", [H, H], F16, kind="ExternalInput")
    W_hhT_d = nc.dram_tensor("W_hhT", [H, 3 * H], F16, kind="ExternalInput")
    W_ihcT_d = nc.dram_tensor("W_ihcT", [E, 3 * H], F16, kind="ExternalInput")
    W_iheT_d = nc.dram_tensor("W_iheT", [E, 3 * H], F16, kind="ExternalInput")
    vw_d = nc.dram_tensor("vw", [H, 1], F16, kind="ExternalInput")
    bsum_d = nc.dram_tensor("bsum", [3 * H, 1], F32, kind="ExternalInput")
    attnb_d = nc.dram_tensor("attnb", [H, 1], F32, kind="ExternalInput")
    fcW_d = nc.dram_tensor("fcW", [H, V], F16, kind="ExternalInput")
    out_d = nc.dram_tensor("out", [T * BL, V], F16, kind="ExternalOutput")
    if has_fcb:
        fcb_d = nc.dram_tensor("fcb", [1, V], F32, kind="ExternalInput")

    r3 = lambda ap: ap.rearrange("(kt p) m -> p kt m", p=128)

    with tile.TileContext(nc) as tc:
        with tc.tile_pool(name="persist", bufs=1) as P1:
            # ---- input DMAs, ordered so the DMA device serves the
            # precompute first, recurrence weights next, fcW last ----
            featsT = P1.tile([128, KT, R, BL], F16)
            nc.sync.dma_start(featsT[:], featsT_d.ap().rearrange(
                "(kt p) r b -> p kt r b", p=128))
            attn_We = P1.tile([128, KT, H], F16)
            nc.sync.dma_start(attn_We[:], r3(attn_We_d.ap()))
            embT = P1.tile([128, KT, T * BL], F16)
            nc.scalar.dma_start(embT[:], r3(embT_d.ap()))
            W_iheT = P1.tile([128, KT, 3 * H], F16)
            nc.scalar.dma_start(W_iheT[:], r3(W_iheT_d.ap()))

            attnb = P1.tile([128, KT, 1], F32)
            nc.gpsimd.dma_start(attnb[:], r3(attnb_d.ap()))
            bsum = P1.tile([128, M3H, 1], F32)
            nc.gpsimd.dma_start(bsum[:], r3(bsum_d.ap()))
            vw = P1.tile([128, KT, 1], F16)
            nc.gpsimd.dma_start(vw[:], r3(vw_d.ap()))
            attn_Wh = P1.tile([128, KT, H], F16)
            nc.gpsimd.dma_start(attn_Wh[:], r3(attn_Wh_d.ap()))
            W_hhT = P1.tile([128, KT, 3 * H], F16)
            nc.gpsimd.dma_start(W_hhT[:], r3(W_hhT_d.ap()))
            W_ihcT = P1.tile([128, KT, 3 * H], F16)
            nc.gpsimd.dma_start(W_ihcT[:], r3(W_ihcT_d.ap()))
            feats16 = P1.tile([128, KT, BL, R], F16)
            nc.gpsimd.dma_start(feats16[:], featsb_d.ap().rearrange(
                "(kt p) b r -> p kt b r", p=128))
            fcW = P1.tile([128, KT, V], F16)
            for kt in range(KT):
                nc.gpsimd.dma_start(fcW[:, kt], r3(fcW_d.ap())[:, kt])
            if has_fcb:
                fcb = P1.tile([128, V], F32)
                nc.gpsimd.dma_start(fcb[:], fcb_d.ap().to_broadcast((128, V)))

            ones16 = P1.tile([1, 128], F16)
            nc.vector.memset(ones16[:], 1.0)
            ones32 = P1.tile([1, 128], F32)
            nc.vector.memset(ones32[:], 1.0)

            # persistent recurrence state / precompute outputs
            fpT = P1.tile([128, KT, R, BL], F16)       # feat_proj + attn_b
            xgxT = P1.tile([128, M3H, T * BL], F32)    # emb-side gate preacts
            h0 = P1.tile([128, KT, BL], F16)
            nc.vector.memset(h0[:], 0.0)
            h_lo = P1.tile([128, KT, 16 * BL], F16)    # h outputs, steps 0..15
            h_hi = P1.tile([128, KT, 16 * BL], F16)    # h outputs, steps 16..31

            # ---- precompute: feat_proj and xgx ----
            with tc.tile_pool(name="pre_ps", bufs=2, space="PSUM") as PPS:
                for mo in range(KT):
                    ps = PPS.tile([128, R * BL], F32, name="fp_ps")
                    for kt in range(KT):
                        nc.tensor.matmul(
                            ps[:], attn_We[:, kt, mo * 128:(mo + 1) * 128],
                            featsT[:, kt].rearrange("p r b -> p (r b)"),
                            start=(kt == 0), stop=(kt == KT - 1))
                    nc.vector.tensor_scalar(
                        out=fpT[:, mo].rearrange("p r b -> p (r b)"),
                        in0=ps[:], scalar1=attnb[:, mo], scalar2=None,
                        op0=OP.add)
                for m in range(M3H):
                    ps = PPS.tile([128, T * BL], F32, name="xg_ps")
                    for kt in range(KT):
                        nc.tensor.matmul(
                            ps[:], W_iheT[:, kt, m * 128:(m + 1) * 128],
                            embT[:, kt], start=(kt == 0), stop=(kt == KT - 1))
                    nc.scalar.add(xgxT[:, m], ps[:], add=bsum[:, m])

            # ---- recurrence + interleaved first-half fc ----
            with tc.tile_pool(name="scratch", bufs=2) as PSC, \
                 tc.tile_pool(name="gates", bufs=2) as PG, \
                 tc.tile_pool(name="ps_hp", bufs=1, space="PSUM") as PS_HP, \
                 tc.tile_pool(name="ps_sc", bufs=1, space="PSUM") as PS_SC, \
                 tc.tile_pool(name="ps_bc", bufs=1, space="PSUM") as PS_BC, \
                 tc.tile_pool(name="ps_g", bufs=1, space="PSUM") as PS_G, \
                 tc.tile_pool(name="fc_ps", bufs=2, space="PSUM") as FPS, \
                 tc.tile_pool(name="fc_sb", bufs=3) as FSB:

                def fc_chunk(mo, ch, qsel):
                    h_src = h_lo if mo == 0 else h_hi
                    rows = slice(mo * 128, (mo + 1) * 128)
                    nv = min(512, V - ch * 512)
                    cols = slice(ch * 512, ch * 512 + nv)
                    ps = FPS.tile([128, 512], F32, name="fc_ps")
                    for kt in range(KT):
                        nc.tensor.matmul(
                            ps[:, :nv], h_src[:, kt], fcW[:, kt, cols],
                            start=(kt == 0), stop=(kt == KT - 1))
                    ot = FSB.tile([128, 512], F16, name="fc_ot")
                    if has_fcb:
                        nc.vector.tensor_tensor(
                            out=ot[:, :nv], in0=ps[:, :nv], in1=fcb[:, cols],
                            op=OP.add)
                    else:
                        nc.scalar.copy(ot[:, :nv], ps[:, :nv])
                    nc.sync.dma_start(out_d.ap()[rows, cols], ot[:, :nv])

                # fc chunk schedule: first-half chunks spread over steps 16..31
                fc_sched = {}
                for s in range(16, T):
                    lo = (s - 16) * NCH // 16
                    hi = (s - 15) * NCH // 16
                    fc_sched[s] = list(range(lo, hi))

                for t in range(T):
                    if t == 0:
                        h_prev = h0[:]
                    elif t <= 16:
                        h_prev = h_lo[:, :, (t - 1) * BL:t * BL]
                    else:
                        h_prev = h_hi[:, :, (t - 17) * BL:(t - 16) * BL]
                    h_slab = h_lo if t < 16 else h_hi
                    hcol = (t % 16) * BL
                    xg = xgxT[:, :, t * BL:(t + 1) * BL]

                    # --- PE: h_proj first (it heads the energy chain) ---
                    hp = PS_HP.tile([128, KT, BL], F32, name="hp")
                    for mo in range(KT):
                        for kt in range(KT):
                            nc.tensor.matmul(
                                hp[:, mo], attn_Wh[:, kt, mo * 128:(mo + 1) * 128],
                                h_prev[:, kt], start=(kt == 0),
                                stop=(kt == KT - 1))
                    # gh fills PE while the energy chain runs
                    g_gh = PS_G.tile([128, M3H, BL], F32, name="g_gh")
                    g_cgx = PS_G.tile([128, M3H, BL], F32, name="g_cgx")
                    for m in range(M3H):
                        for kt in range(KT):
                            nc.tensor.matmul(
                                g_gh[:, m], W_hhT[:, kt, m * 128:(m + 1) * 128],
                                h_prev[:, kt], start=(kt == 0),
                                stop=(kt == KT - 1))

                    # --- energy: tanh(fp + hp) then scores, in two r-halves
                    hp16 = PSC.tile([128, KT, BL], F16, name="hp16")
                    nc.vector.tensor_copy(hp16[:], hp[:])
                    sc = PS_SC.tile([1, R, BL], F32, name="sc", bufs=1)
                    en_b = PSC.tile([128, KT, R, BL], F16, name="en_b", bufs=1)
                    for (r0, r1) in RSPLIT:
                        nr = r1 - r0
                        en_f = PSC.tile([128, KT, 25, BL], F16,
                                        name=f"en_f{r0}", bufs=1)
                        nc.vector.tensor_tensor(
                            out=en_f[:, :, :nr], in0=fpT[:, :, r0:r1],
                            in1=hp16[:, :, None, :].to_broadcast(
                                (128, KT, nr, BL)),
                            op=OP.add)
                        nc.scalar.activation(
                            en_b[:, :, r0:r1], en_f[:, :, :nr], AF.Tanh)
                        for kt in range(KT):
                            nc.tensor.matmul(
                                sc[:, r0:r1].rearrange("p r b -> p (r b)"),
                                vw[:, kt],
                                en_b[:, kt, r0:r1].rearrange("p r b -> p (r b)"),
                                start=(kt == 0), stop=(kt == KT - 1))

                    # gate pre-adds that only need gh + xg: emitted after the
                    # energy chain so DVE prioritizes the chain
                    rzpre = PG.tile([128, 8, BL], F32, name="rzpre")
                    nc.vector.tensor_tensor(
                        out=rzpre[:], in0=g_gh[:, 0:8], in1=xg[:, 0:8],
                        op=OP.add)
                    # npre2 = xg_n + 0.5*gh_n  (n = tanh(npre2 + cgx_n
                    #                            + 0.5*t_r*gh_n - 0.5*gh_n
                    #   ... with r = 0.5 + 0.5*t_r: r*hn = 0.5*hn + 0.5*t_r*hn)
                    npre2 = PG.tile([128, 4, BL], F32, name="npre2")
                    nc.vector.scalar_tensor_tensor(
                        out=npre2[:], in0=g_gh[:, 8:12], scalar=0.5,
                        in1=xg[:, 8:12], op0=OP.mult, op1=OP.add)

                    # softmax (unnormalized; scores are O(1), no max-sub)
                    ex = PSC.tile([1, BL, R], F16, name="ex")
                    nc.scalar.activation(
                        ex[:].rearrange("p b r -> p r b"), sc[:], AF.Exp)
                    s_sum = PSC.tile([1, BL], F32, name="s_sum")
                    nc.vector.tensor_reduce(
                        out=s_sum[:], in_=ex[:],
                        axis=mybir.AxisListType.X, op=OP.add)
                    rec = PSC.tile([1, BL], F32, name="rec")
                    nc.vector.reciprocal(rec[:], s_sum[:])
                    # replicate ex and rec across partitions via PE rank-1
                    exb_ps = PS_BC.tile([128, BL * R], F32, name="exb_ps")
                    nc.tensor.matmul(
                        exb_ps[:], ones16[:], ex[:].rearrange("p b r -> p (b r)"),
                        start=True, stop=True)
                    recb_ps = PS_BC.tile([128, BL], F32, name="recb_ps")
                    nc.tensor.matmul(recb_ps[:], ones32[:], rec[:],
                                     start=True, stop=True)
                    # PSUM->SBUF copies on ACT so DVE stays on the chain
                    exb = PSC.tile([128, BL, R], F16, name="exb", bufs=1)
                    nc.scalar.copy(exb[:].rearrange("p b r -> p (b r)"), exb_ps[:])
                    recb = PSC.tile([128, BL], F32, name="recb")
                    nc.scalar.copy(recb[:], recb_ps[:])

                    # context = (sum_r exb * feats) * recb
                    prod = PSC.tile([128, KT, BL, R], F16, name="prod", bufs=1)
                    nc.vector.tensor_tensor(
                        out=prod[:], in0=feats16[:],
                        in1=exb[:, None].to_broadcast((128, KT, BL, R)),
                        op=OP.mult)
                    ctx_u = PSC.tile([128, KT, BL], F32, name="ctx_u")
                    nc.vector.tensor_reduce(
                        out=ctx_u[:], in_=prod[:],
                        axis=mybir.AxisListType.X, op=OP.add)
                    ctx16 = PSC.tile([128, KT, BL], F16, name="ctx16")
                    nc.vector.tensor_tensor(
                        out=ctx16[:], in0=ctx_u[:],
                        in1=recb[:, None, :].to_broadcast((128, KT, BL)),
                        op=OP.mult)

                    # cgx = W_ih[:, E:] @ context
                    for m in range(M3H):
                        for kt in range(KT):
                            nc.tensor.matmul(
                                g_cgx[:, m], W_ihcT[:, kt, m * 128:(m + 1) * 128],
                                ctx16[:, kt], start=(kt == 0),
                                stop=(kt == KT - 1))

                    # --- gates (sigmoid = 0.5 + 0.5*tanh(x/2), kept as raw
                    # t = tanh(x/2) and folded algebraically) ---
                    rz_t = PG.tile([128, 8, BL], F16, name="rz_t")
                    nc.vector.tensor_tensor(
                        out=rz_t[:], in0=g_cgx[:, 0:8], in1=rzpre[:],
                        op=OP.add)
                    t_rz = PG.tile([128, 8, BL], F16, name="t_rz")
                    nc.scalar.activation(t_rz[:], rz_t[:], AF.Tanh, scale=0.5)
                    # tmp_n = npre2 + cgx_n  (during the t_rz activation)
                    tmp_n = PG.tile([128, 4, BL], F32, name="tmp_n")
                    nc.vector.tensor_tensor(
                        out=tmp_n[:], in0=npre2[:], in1=g_cgx[:, 8:12],
                        op=OP.add)
                    # n = tanh(tmp_n + 0.5*t_r*gh_n)
                    q = PG.tile([128, 4, BL], F32, name="q")
                    nc.vector.tensor_tensor(
                        out=q[:], in0=t_rz[:, 0:4], in1=g_gh[:, 8:12],
                        op=OP.mult)
                    n_p = PG.tile([128, 4, BL], F16, name="n_p")
                    nc.vector.scalar_tensor_tensor(
                        out=n_p[:], in0=q[:], scalar=0.5, in1=tmp_n[:],
                        op0=OP.mult, op1=OP.add)
                    n_t = PG.tile([128, 4, BL], F16, name="n_t")
                    nc.scalar.activation(n_t[:], n_p[:], AF.Tanh)
                    # h_new = n + z*(h-n), z = 0.5 + 0.5*t_z:
                    #   d = h - n; u = (t_z + 1)*d; h_new = 0.5*u + n
                    d = PG.tile([128, 4, BL], F16, name="d")
                    nc.vector.tensor_tensor(
                        out=d[:], in0=h_prev, in1=n_t[:], op=OP.subtract)
                    u = PG.tile([128, 4, BL], F16, name="u")
                    nc.vector.scalar_tensor_tensor(
                        out=u[:], in0=t_rz[:, 4:8], scalar=1.0, in1=d[:],
                        op0=OP.add, op1=OP.mult)
                    nc.vector.scalar_tensor_tensor(
                        out=h_slab[:, :, hcol:hcol + BL], in0=u[:], scalar=0.5,
                        in1=n_t[:], op0=OP.mult, op1=OP.add)

                    # interleave first-half fc chunks on the idle PE
                    for ch in fc_sched.get(t, ()):
                        fc_chunk(0, ch, ch)

                # ---- tail: second-half fc ----
                for ch in range(NCH):
                    fc_chunk(1, ch, ch)

    nc.compile()
    return nc


def _get_built(has_fcb=True):
    with _BUILD_LOCK:
        if has_fcb not in _BUILT:
            _BUILT[has_fcb] = _build(has_fcb)
    return _BUILT[has_fcb]


def kernel(features, captions, embed_table, attn_W, attn_b, v_w,
           W_ih, W_hh, b_ih, b_hh, fc_W, fc_b):
    from concourse.bass_utils import run_bass_kernel_spmd

    features = np.asarray(features, dtype=np.float32)
    captions = np.asarray(captions)
    embed_table = np.asarray(embed_table, dtype=np.float32)
    attn_W = np.asarray(attn_W, dtype=np.float32)
    attn_b = np.asarray(attn_b, dtype=np.float32)
    v_w = np.asarray(v_w, dtype=np.float32)
    W_ih = np.asarray(W_ih, dtype=np.float32)
    W_hh = np.asarray(W_hh, dtype=np.float32)
    b_ih = np.asarray(b_ih, dtype=np.float32)
    b_hh = np.asarray(b_hh, dtype=np.float32)
    fc_W = np.asarray(fc_W, dtype=np.float32)
    fc_b = np.asarray(fc_b, dtype=np.float32)

    has_fcb = bool(np.any(fc_b))
    nc = _get_built(has_fcb)

    f16 = np.float16
    shared = {
        "attn_We": attn_W[:E].astype(f16),
        "attn_Wh": attn_W[E:].astype(f16),
        "W_hhT": np.ascontiguousarray(W_hh.T).astype(f16),
        "W_ihcT": np.ascontiguousarray(W_ih[:, E:].T).astype(f16),
        "W_iheT": np.ascontiguousarray(W_ih[:, :E].T).astype(f16),
        "vw": v_w[:, None].astype(f16),
        "bsum": np.ascontiguousarray((b_ih + b_hh)[:, None]),
        "attnb": np.ascontiguousarray(attn_b[:, None]),
        "fcW": fc_W.astype(f16),
    }
    if has_fcb:
        shared["fcb"] = np.ascontiguousarray(fc_b[None, :])
    emb = embed_table[captions[:, :T].astype(np.int64)]  # [B, T, E]
    in_maps = []
    for c in range(NCORES):
        rows = slice(c * BL, (c + 1) * BL)
        m = dict(shared)
        m["featsT"] = features[rows].transpose(2, 1, 0).astype(f16)
        m["featsb"] = features[rows].transpose(2, 0, 1).astype(f16)
        m["embT"] = emb[rows].transpose(2, 1, 0).reshape(E, T * BL).astype(f16)
        in_maps.append(m)

    res = run_bass_kernel_spmd(nc, in_maps, core_ids=list(range(NCORES)))

    out = np.empty((B, T, V), dtype=np.float32)
    for c in range(NCORES):
        # rows of per-core output are t*BL + b_local
        out[c * BL:(c + 1) * BL] = (
            res.results[c]["out"].astype(np.float32)
            .reshape(T, BL, V).transpose(1, 0, 2))
    return out


# revision 13
# speedup vs baseline: 1.0496x; 1.0496x over previous
"""Trainium2 Bass kernel for nn_DecoderGRU (attention GRU decoder + vocab head).

Strategy (8 NeuronCores, data-parallel over batch, 8 rows/core):
  - All-fp16 tensors (weights, activations, history): halves DMA traffic and
    doubles DVE throughput where 2x modes apply; PSUM accumulation is fp32.
  - Startup: precompute inputs (featsT/attn_We/embT/W_iheT) DMA'd first,
    recurrence weights next, fcW last so the recurrence starts ~20us in.
  - Hoisted out of the 32-step loop: feat_proj, xgx (= emb @ W_ih[:, :E].T
    + b_ih + b_hh), and the fc head.
  - fc head: stationary = 128 finished h columns; the first half (steps
    0..15) is interleaved into steps 16..31 on the otherwise idle PE, with
    logits DMA'd straight from PSUM to DRAM; only the second half runs as a
    tail after the loop.
  - Per step, the serial chain is minimized: hp before gh on PE; gate
    pre-adds emitted after the energy chain; exb/recb PSUM->SBUF copies on
    ACT (frees DVE); sigmoid via raw tanh(x/2) algebra with
    scalar_tensor_tensor fusions; h_new written directly into the fp16
    history slab used by both the next step and the fc head.
"""

import threading

import numpy as np

B, R, E, H, V, L = 64, 49, 512, 512, 10000, 33
T = L - 1            # 32 decode steps
NCORES = 8
BL = B // NCORES     # 8 batch rows per core
KT = E // 128        # 4 k-tiles of 128 for E=H=512
M3H = (3 * H) // 128  # 12 m-tiles for gate dim
RSPLIT = ((0, 25), (25, 49))  # r-halves for the energy pipeline
NCH = (V + 511) // 512        # 20 vocab chunks, last = 272

_BUILD_LOCK = threading.Lock()
_BUILT = {}


def _build(has_fcb=True):
    import concourse.mybir as mybir
    import concourse.tile as tile
    from concourse import bacc

    F32 = mybir.dt.float32
    F16 = mybir.dt.float16
    AF = mybir.ActivationFunctionType
    OP = mybir.AluOpType

    nc = bacc.Bacc("TRN2", target_bir_lowering=False, debug=False,
                   num_devices=NCORES)

    # ---- DRAM I/O (everything fp16 except f32 biases and the output) ----
    featsT_d = nc.dram_tensor("featsT", [E, R, BL], F16, kind="ExternalInput")
    featsb_d = nc.dram_tensor("featsb", [E, BL, R], F16, kind="ExternalInput")
    embT_d = nc.dram_tensor("embT", [E, T * BL], F16, kind="ExternalInput")
    attn_We_d = nc.dram_tensor("attn_We", [E, H], F16, kind="ExternalInput")
    attn_Wh_d = nc.dram_tensor("attn_Wh", [H, H], F16, kind="ExternalInput")
    W_hhT_d = nc.dram_tensor("W_hhT", [H, 3 * H], F16, kind="ExternalInput")
    W_ihcT_d = nc.dram_tensor("W_ihcT", [E, 3 * H], F16, kind="ExternalInput")
    W_iheT_d = nc.dram_tensor("W_iheT", [E, 3 * H], F16, kind="ExternalInput")
    vw_d = nc.dram_tensor("vw", [H, 128], F16, kind="ExternalInput")
    bsum_d = nc.dram_tensor("bsum", [3 * H, 1], F32, kind="ExternalInput")
    attnb_d = nc.dram_tensor("attnb", [H, 1], F32, kind="ExternalInput")
    fcW_d = nc.dram_tensor("fcW", [H, V], F16, kind="ExternalInput")
    out_d = nc.dram_tensor("out", [T * BL, V], F16, kind="ExternalOutput")
    if has_fcb:
        fcb_d = nc.dram_tensor("fcb", [1, V], F32, kind="ExternalInput")

    r3 = lambda ap: ap.rearrange("(kt p) m -> p kt m", p=128)

    with tile.TileContext(nc) as tc:
        with tc.tile_pool(name="persist", bufs=1) as P1:
            # ---- input DMAs, ordered so the DMA device serves the
            # precompute first, recurrence weights next, fcW last ----
            featsT = P1.tile([128, KT, R, BL], F16)
            nc.sync.dma_start(featsT[:], featsT_d.ap().rearrange(
                "(kt p) r b -> p kt r b", p=128))
            attn_We = P1.tile([128, KT, H], F16)
            nc.sync.dma_start(attn_We[:], r3(attn_We_d.ap()))
            embT = P1.tile([128, KT, T * BL], F16)
            nc.scalar.dma_start(embT[:], r3(embT_d.ap()))
            W_iheT = P1.tile([128, KT, 3 * H], F16)
            nc.scalar.dma_start(W_iheT[:], r3(W_iheT_d.ap()))

            attnb = P1.tile([128, KT, 1], F32)
            nc.gpsimd.dma_start(attnb[:], r3(attnb_d.ap()))
            bsum = P1.tile([128, M3H, 1], F32)
            nc.gpsimd.dma_start(bsum[:], r3(bsum_d.ap()))
            vw = P1.tile([128, KT, 128], F16)
            nc.gpsimd.dma_start(vw[:], r3(vw_d.ap()))
            attn_Wh = P1.tile([128, KT, H], F16)
            nc.gpsimd.dma_start(attn_Wh[:], r3(attn_Wh_d.ap()))
            W_hhT = P1.tile([128, KT, 3 * H], F16)
            nc.gpsimd.dma_start(W_hhT[:], r3(W_hhT_d.ap()))
            W_ihcT = P1.tile([128, KT, 3 * H], F16)
            nc.gpsimd.dma_start(W_ihcT[:], r3(W_ihcT_d.ap()))
            feats16 = P1.tile([128, KT, BL, R], F16)
            nc.gpsimd.dma_start(feats16[:], featsb_d.ap().rearrange(
                "(kt p) b r -> p kt b r", p=128))
            fcW = P1.tile([128, KT, V], F16)
            for kt in range(KT):
                nc.gpsimd.dma_start(fcW[:, kt], r3(fcW_d.ap())[:, kt])
            if has_fcb:
                fcb = P1.tile([128, V], F32)
                nc.gpsimd.dma_start(fcb[:], fcb_d.ap().to_broadcast((128, V)))

            # persistent recurrence state / precompute outputs
            fpT = P1.tile([128, KT, R, BL], F16)       # feat_proj + attn_b
            xgxT = P1.tile([128, M3H, T * BL], F32)    # emb-side gate preacts
            h0 = P1.tile([128, KT, BL], F16)
            nc.vector.memset(h0[:], 0.0)
            h_lo = P1.tile([128, KT, 16 * BL], F16)    # h outputs, steps 0..15
            h_hi = P1.tile([128, KT, 16 * BL], F16)    # h outputs, steps 16..31

            # ---- precompute: feat_proj and xgx ----
            with tc.tile_pool(name="pre_ps", bufs=2, space="PSUM") as PPS:
                for mo in range(KT):
                    ps = PPS.tile([128, R * BL], F32, name="fp_ps")
                    for kt in range(KT):
                        nc.tensor.matmul(
                            ps[:], attn_We[:, kt, mo * 128:(mo + 1) * 128],
                            featsT[:, kt].rearrange("p r b -> p (r b)"),
                            start=(kt == 0), stop=(kt == KT - 1))
                    nc.vector.tensor_scalar(
                        out=fpT[:, mo].rearrange("p r b -> p (r b)"),
                        in0=ps[:], scalar1=attnb[:, mo], scalar2=None,
                        op0=OP.add)
                for m in range(M3H):
                    ps = PPS.tile([128, T * BL], F32, name="xg_ps")
                    for kt in range(KT):
                        nc.tensor.matmul(
                            ps[:], W_iheT[:, kt, m * 128:(m + 1) * 128],
                            embT[:, kt], start=(kt == 0), stop=(kt == KT - 1))
                    nc.scalar.add(xgxT[:, m], ps[:], add=bsum[:, m])

            # ---- recurrence + interleaved first-half fc ----
            with tc.tile_pool(name="scratch", bufs=2) as PSC, \
                 tc.tile_pool(name="gates", bufs=2) as PG, \
                 tc.tile_pool(name="ps_hp", bufs=1, space="PSUM") as PS_HP, \
                 tc.tile_pool(name="ps_sc", bufs=1, space="PSUM") as PS_SC, \
                 tc.tile_pool(name="ps_g", bufs=1, space="PSUM") as PS_G, \
                 tc.tile_pool(name="fc_ps", bufs=3, space="PSUM") as FPS, \
                 tc.tile_pool(name="fc_sb", bufs=3) as FSB:

                def fc_chunk(mo, ch, qsel):
                    h_src = h_lo if mo == 0 else h_hi
                    rows = slice(mo * 128, (mo + 1) * 128)
                    nv = min(512, V - ch * 512)
                    cols = slice(ch * 512, ch * 512 + nv)
                    ps = FPS.tile([128, 512], F32, name="fc_ps")
                    for kt in range(KT):
                        nc.tensor.matmul(
                            ps[:, :nv], h_src[:, kt], fcW[:, kt, cols],
                            start=(kt == 0), stop=(kt == KT - 1))
                    ot = FSB.tile([128, 512], F16, name="fc_ot")
                    if has_fcb:
                        nc.vector.tensor_tensor(
                            out=ot[:, :nv], in0=ps[:, :nv], in1=fcb[:, cols],
                            op=OP.add)
                    else:
                        nc.scalar.copy(ot[:, :nv], ps[:, :nv])
                    nc.sync.dma_start(out_d.ap()[rows, cols], ot[:, :nv])

                # fc chunk schedule: first-half chunks spread over steps 16..31
                fc_sched = {}
                for s in range(16, T):
                    lo = (s - 16) * NCH // 16
                    hi = (s - 15) * NCH // 16
                    fc_sched[s] = list(range(lo, hi))

                for t in range(T):
                    if t == 0:
                        h_prev = h0[:]
                    elif t <= 16:
                        h_prev = h_lo[:, :, (t - 1) * BL:t * BL]
                    else:
                        h_prev = h_hi[:, :, (t - 17) * BL:(t - 16) * BL]
                    h_slab = h_lo if t < 16 else h_hi
                    hcol = (t % 16) * BL
                    xg = xgxT[:, :, t * BL:(t + 1) * BL]

                    # --- PE: h_proj first (it heads the energy chain) ---
                    hp = PS_HP.tile([128, KT, BL], F32, name="hp")
                    for mo in range(KT):
                        for kt in range(KT):
                            nc.tensor.matmul(
                                hp[:, mo], attn_Wh[:, kt, mo * 128:(mo + 1) * 128],
                                h_prev[:, kt], start=(kt == 0),
                                stop=(kt == KT - 1))
                    # gh fills PE while the energy chain runs
                    g_gh = PS_G.tile([128, M3H, BL], F32, name="g_gh")
                    g_cgx = PS_G.tile([128, M3H, BL], F32, name="g_cgx")
                    for m in range(M3H):
                        for kt in range(KT):
                            nc.tensor.matmul(
                                g_gh[:, m], W_hhT[:, kt, m * 128:(m + 1) * 128],
                                h_prev[:, kt], start=(kt == 0),
                                stop=(kt == KT - 1))

                    # --- energy: tanh(fp + hp) then scores, in two r-halves
                    hp16 = PSC.tile([128, KT, BL], F16, name="hp16")
                    nc.vector.tensor_copy(hp16[:], hp[:])
                    # scores land replicated on all 128 partitions (vw is
                    # column-replicated), so softmax needs no broadcast
                    sc = PS_SC.tile([128, R, BL], F32, name="sc", bufs=1)
                    en_b = PSC.tile([128, KT, R, BL], F16, name="en_b", bufs=1)
                    for (r0, r1) in RSPLIT:
                        nr = r1 - r0
                        en_f = PSC.tile([128, KT, 25, BL], F16,
                                        name=f"en_f{r0}", bufs=1)
                        nc.vector.tensor_tensor(
                            out=en_f[:, :, :nr], in0=fpT[:, :, r0:r1],
                            in1=hp16[:, :, None, :].to_broadcast(
                                (128, KT, nr, BL)),
                            op=OP.add)
                        nc.scalar.activation(
                            en_b[:, :, r0:r1], en_f[:, :, :nr], AF.Tanh)
                        for kt in range(KT):
                            nc.tensor.matmul(
                                sc[:, r0:r1].rearrange("p r b -> p (r b)"),
                                vw[:, kt],
                                en_b[:, kt, r0:r1].rearrange("p r b -> p (r b)"),
                                start=(kt == 0), stop=(kt == KT - 1))

                    # gate pre-adds that only need gh + xg: emitted after the
                    # energy chain so DVE prioritizes the chain
                    rzpre = PG.tile([128, 8, BL], F32, name="rzpre")
                    nc.vector.tensor_tensor(
                        out=rzpre[:], in0=g_gh[:, 0:8], in1=xg[:, 0:8],
                        op=OP.add)
                    # npre2 = xg_n + 0.5*gh_n  (n = tanh(npre2 + cgx_n
                    #                            + 0.5*t_r*gh_n - 0.5*gh_n
                    #   ... with r = 0.5 + 0.5*t_r: r*hn = 0.5*hn + 0.5*t_r*hn)
                    npre2 = PG.tile([128, 4, BL], F32, name="npre2")
                    nc.vector.scalar_tensor_tensor(
                        out=npre2[:], in0=g_gh[:, 8:12], scalar=0.5,
                        in1=xg[:, 8:12], op0=OP.mult, op1=OP.add)

                    # softmax (unnormalized; scores are O(1), no max-sub):
                    # exp reads replicated scores from PSUM, writes replicated
                    # fp16 weights straight to SBUF
                    exb = PSC.tile([128, BL, R], F16, name="exb", bufs=1)
                    nc.scalar.activation(
                        exb[:].rearrange("p b r -> p r b"), sc[:], AF.Exp)
                    s_sum = PSC.tile([128, BL], F32, name="s_sum")
                    nc.vector.tensor_reduce(
                        out=s_sum[:], in_=exb[:],
                        axis=mybir.AxisListType.X, op=OP.add)
                    recb = PSC.tile([128, BL], F32, name="recb")
                    nc.vector.reciprocal(recb[:], s_sum[:])

                    # context = (sum_r exb * feats) * recb; the r-reduction is
                    # two-stage: fold r-halves with a 2x fp16 add, then reduce
                    prod = PSC.tile([128, KT, BL, R], F16, name="prod", bufs=1)
                    nc.vector.tensor_tensor(
                        out=prod[:], in0=feats16[:],
                        in1=exb[:, None].to_broadcast((128, KT, BL, R)),
                        op=OP.mult)
                    fold = PSC.tile([128, KT, BL, 25], F16, name="fold")
                    nc.vector.tensor_tensor(
                        out=fold[:, :, :, 0:24], in0=prod[:, :, :, 0:24],
                        in1=prod[:, :, :, 25:49], op=OP.add)
                    nc.vector.tensor_copy(fold[:, :, :, 24:25],
                                          prod[:, :, :, 24:25])
                    ctx_u = PSC.tile([128, KT, BL], F32, name="ctx_u")
                    nc.vector.tensor_reduce(
                        out=ctx_u[:], in_=fold[:],
                        axis=mybir.AxisListType.X, op=OP.add)
                    ctx16 = PSC.tile([128, KT, BL], F16, name="ctx16")
                    nc.vector.tensor_tensor(
                        out=ctx16[:], in0=ctx_u[:],
                        in1=recb[:, None, :].to_broadcast((128, KT, BL)),
                        op=OP.mult)

                    # cgx = W_ih[:, E:] @ context
                    for m in range(M3H):
                        for kt in range(KT):
                            nc.tensor.matmul(
                                g_cgx[:, m], W_ihcT[:, kt, m * 128:(m + 1) * 128],
                                ctx16[:, kt], start=(kt == 0),
                                stop=(kt == KT - 1))

                    # --- gates (sigmoid = 0.5 + 0.5*tanh(x/2), kept as raw
                    # t = tanh(x/2) and folded algebraically) ---
                    rz_t = PG.tile([128, 8, BL], F16, name="rz_t")
                    nc.vector.tensor_tensor(
                        out=rz_t[:], in0=g_cgx[:, 0:8], in1=rzpre[:],
                        op=OP.add)
                    t_rz = PG.tile([128, 8, BL], F16, name="t_rz")
                    nc.scalar.activation(t_rz[:], rz_t[:], AF.Tanh, scale=0.5)
                    # tmp_n = npre2 + cgx_n  (during the t_rz activation)
                    tmp_n = PG.tile([128, 4, BL], F32, name="tmp_n")
                    nc.vector.tensor_tensor(
                        out=tmp_n[:], in0=npre2[:], in1=g_cgx[:, 8:12],
                        op=OP.add)
                    # n = tanh(tmp_n + 0.5*t_r*gh_n)
                    q = PG.tile([128, 4, BL], F32, name="q")
                    nc.vector.tensor_tensor(
                        out=q[:], in0=t_rz[:, 0:4], in1=g_gh[:, 8:12],
                        op=OP.mult)
                    n_p = PG.tile([128, 4, BL], F16, name="n_p")
                    nc.vector.scalar_tensor_tensor(
                        out=n_p[:], in0=q[:], scalar=0.5, in1=tmp_n[:],
                        op0=OP.mult, op1=OP.add)
                    n_t = PG.tile([128, 4, BL], F16, name="n_t")
                    nc.scalar.activation(n_t[:], n_p[:], AF.Tanh)
                    # h_new = n + z*(h-n), z = 0.5 + 0.5*t_z:
                    #   d = h - n; u = (t_z + 1)*d; h_new = 0.5*u + n
                    d = PG.tile([128, 4, BL], F16, name="d")
                    nc.vector.tensor_tensor(
                        out=d[:], in0=h_prev, in1=n_t[:], op=OP.subtract)
                    u = PG.tile([128, 4, BL], F16, name="u")
                    nc.vector.scalar_tensor_tensor(
                        out=u[:], in0=t_rz[:, 4:8], scalar=1.0, in1=d[:],
                        op0=OP.add, op1=OP.mult)
                    nc.vector.scalar_tensor_tensor(
                        out=h_slab[:, :, hcol:hcol + BL], in0=u[:], scalar=0.5,
                        in1=n_t[:], op0=OP.mult, op1=OP.add)

                    # interleave first-half fc chunks on the idle PE
                    for ch in fc_sched.get(t, ()):
                        fc_chunk(0, ch, ch)

                # ---- tail: second-half fc ----
                for ch in range(NCH):
                    fc_chunk(1, ch, ch)

    nc.compile()
    return nc


def _get_built(has_fcb=True):
    with _BUILD_LOCK:
        if has_fcb not in _BUILT:
            _BUILT[has_fcb] = _build(has_fcb)
    return _BUILT[has_fcb]


def kernel(features, captions, embed_table, attn_W, attn_b, v_w,
           W_ih, W_hh, b_ih, b_hh, fc_W, fc_b):
    from concourse.bass_utils import run_bass_kernel_spmd

    features = np.asarray(features, dtype=np.float32)
    captions = np.asarray(captions)
    embed_table = np.asarray(embed_table, dtype=np.float32)
    attn_W = np.asarray(attn_W, dtype=np.float32)
    attn_b = np.asarray(attn_b, dtype=np.float32)
    v_w = np.asarray(v_w, dtype=np.float32)
    W_ih = np.asarray(W_ih, dtype=np.float32)
    W_hh = np.asarray(W_hh, dtype=np.float32)
    b_ih = np.asarray(b_ih, dtype=np.float32)
    b_hh = np.asarray(b_hh, dtype=np.float32)
    fc_W = np.asarray(fc_W, dtype=np.float32)
    fc_b = np.asarray(fc_b, dtype=np.float32)

    has_fcb = bool(np.any(fc_b))
    nc = _get_built(has_fcb)

    f16 = np.float16
    shared = {
        "attn_We": attn_W[:E].astype(f16),
        "attn_Wh": attn_W[E:].astype(f16),
        "W_hhT": np.ascontiguousarray(W_hh.T).astype(f16),
        "W_ihcT": np.ascontiguousarray(W_ih[:, E:].T).astype(f16),
        "W_iheT": np.ascontiguousarray(W_ih[:, :E].T).astype(f16),
        "vw": np.repeat(v_w[:, None], 128, axis=1).astype(f16),
        "bsum": np.ascontiguousarray((b_ih + b_hh)[:, None]),
        "attnb": np.ascontiguousarray(attn_b[:, None]),
        "fcW": fc_W.astype(f16),
    }
    if has_fcb:
        shared["fcb"] = np.ascontiguousarray(fc_b[None, :])
    emb = embed_table[captions[:, :T].astype(np.int64)]  # [B, T, E]
    in_maps = []
    for c in range(NCORES):
        rows = slice(c * BL, (c + 1) * BL)
        m = dict(shared)
        m["featsT"] = features[rows].transpose(2, 1, 0).astype(f16)
        m["featsb"] = features[rows].transpose(2, 0, 1).astype(f16)
        m["embT"] = emb[rows].transpose(2, 1, 0).reshape(E, T * BL).astype(f16)
        in_maps.append(m)

    res = run_bass_kernel_spmd(nc, in_maps, core_ids=list(range(NCORES)))

    out = np.empty((B, T, V), dtype=np.float32)
    for c in range(NCORES):
        # rows of per-core output are t*BL + b_local
        out[c * BL:(c + 1) * BL] = (
            res.results[c]["out"].astype(np.float32)
            .reshape(T, BL, V).transpose(1, 0, 2))
    return out


# revision 19
# speedup vs baseline: 1.0640x; 1.0137x over previous
"""Trainium2 Bass kernel for nn_DecoderGRU (attention GRU decoder + vocab head).

Strategy (8 NeuronCores, data-parallel over batch, 8 rows/core):
  - All-fp16 tensors (weights, activations, history): halves DMA traffic and
    doubles DVE throughput where 2x modes apply; PSUM accumulation is fp32.
  - Startup: precompute inputs (featsT/attn_We/embT/W_iheT) DMA'd first,
    recurrence weights next, fcW last so the recurrence starts ~20us in.
  - Hoisted out of the 32-step loop: feat_proj, xgx (= emb @ W_ih[:, :E].T
    + b_ih + b_hh), and the fc head.
  - fc head: stationary = 128 finished h columns; the first half (steps
    0..15) is interleaved into steps 16..31 on the otherwise idle PE, with
    logits DMA'd straight from PSUM to DRAM; only the second half runs as a
    tail after the loop.
  - Per step, the serial chain is minimized: hp before gh on PE; gate
    pre-adds emitted after the energy chain; exb/recb PSUM->SBUF copies on
    ACT (frees DVE); sigmoid via raw tanh(x/2) algebra with
    scalar_tensor_tensor fusions; h_new written directly into the fp16
    history slab used by both the next step and the fc head.
"""

import threading

import numpy as np

B, R, E, H, V, L = 64, 49, 512, 512, 10000, 33
T = L - 1            # 32 decode steps
NCORES = 8
BL = B // NCORES     # 8 batch rows per core
KT = E // 128        # 4 k-tiles of 128 for E=H=512
M3H = (3 * H) // 128  # 12 m-tiles for gate dim
RSPLIT = ((0, 25), (25, 49))  # r-halves for the energy pipeline
NCH = (V + 511) // 512        # 20 vocab chunks, last = 272

_BUILD_LOCK = threading.Lock()
_BUILT = {}


def _build(has_fcb=True):
    import concourse.mybir as mybir
    import concourse.tile as tile
    from concourse import bacc

    F32 = mybir.dt.float32
    F16 = mybir.dt.float16
    AF = mybir.ActivationFunctionType
    OP = mybir.AluOpType

    nc = bacc.Bacc("TRN2", target_bir_lowering=False, debug=False,
                   num_devices=NCORES)

    # ---- DRAM I/O (everything fp16 except f32 biases and the output) ----
    featsT_d = nc.dram_tensor("featsT", [E, R, BL], F16, kind="ExternalInput")
    featsb_d = nc.dram_tensor("featsb", [E, BL, R], F16, kind="ExternalInput")
    embT_d = nc.dram_tensor("embT", [E, T * BL], F16, kind="ExternalInput")
    attn_We_d = nc.dram_tensor("attn_We", [E, H], F16, kind="ExternalInput")
    attn_Wh_d = nc.dram_tensor("attn_Wh", [H, H], F16, kind="ExternalInput")
    W_hhT_d = nc.dram_tensor("W_hhT", [H, 3 * H], F16, kind="ExternalInput")
    W_ihcT_d = nc.dram_tensor("W_ihcT", [E, 3 * H], F16, kind="ExternalInput")
    W_iheT_d = nc.dram_tensor("W_iheT", [E, 3 * H], F16, kind="ExternalInput")
    vw_d = nc.dram_tensor("vw", [H, 128], F16, kind="ExternalInput")
    bsum_d = nc.dram_tensor("bsum", [3 * H, 1], F32, kind="ExternalInput")
    attnb_d = nc.dram_tensor("attnb", [H, 1], F32, kind="ExternalInput")
    fcW_d = nc.dram_tensor("fcW", [H, V], F16, kind="ExternalInput")
    out_d = nc.dram_tensor("out", [T * BL, V], F16, kind="ExternalOutput")
    if has_fcb:
        fcb_d = nc.dram_tensor("fcb", [1, V], F32, kind="ExternalInput")

    r3 = lambda ap: ap.rearrange("(kt p) m -> p kt m", p=128)

    with tile.TileContext(nc) as tc:
        with tc.tile_pool(name="persist", bufs=1) as P1:
            # ---- input DMAs, ordered so the DMA device serves the
            # precompute first, recurrence weights next, fcW last ----
            featsT = P1.tile([128, KT, R, BL], F16)
            nc.sync.dma_start(featsT[:], featsT_d.ap().rearrange(
                "(kt p) r b -> p kt r b", p=128))
            attn_We = P1.tile([128, KT, H], F16)
            nc.sync.dma_start(attn_We[:], r3(attn_We_d.ap()))
            embT = P1.tile([128, KT, T * BL], F16)
            nc.scalar.dma_start(embT[:], r3(embT_d.ap()))
            W_iheT = P1.tile([128, KT, 3 * H], F16)
            nc.scalar.dma_start(W_iheT[:], r3(W_iheT_d.ap()))

            attnb = P1.tile([128, KT, 1], F32)
            nc.gpsimd.dma_start(attnb[:], r3(attnb_d.ap()))
            bsum = P1.tile([128, M3H, 1], F32)
            nc.gpsimd.dma_start(bsum[:], r3(bsum_d.ap()))
            vw = P1.tile([128, KT, 128], F16)
            nc.gpsimd.dma_start(vw[:], r3(vw_d.ap()))
            attn_Wh = P1.tile([128, KT, H], F16)
            nc.gpsimd.dma_start(attn_Wh[:], r3(attn_Wh_d.ap()))
            W_hhT = P1.tile([128, KT, 3 * H], F16)
            nc.gpsimd.dma_start(W_hhT[:], r3(W_hhT_d.ap()))
            W_ihcT = P1.tile([128, KT, 3 * H], F16)
            nc.gpsimd.dma_start(W_ihcT[:], r3(W_ihcT_d.ap()))
            feats16 = P1.tile([128, KT, BL, R], F16)
            nc.gpsimd.dma_start(feats16[:], featsb_d.ap().rearrange(
                "(kt p) b r -> p kt b r", p=128))
            fcW = P1.tile([128, KT, V], F16)
            for kt in range(KT):
                nc.gpsimd.dma_start(fcW[:, kt], r3(fcW_d.ap())[:, kt])
            if has_fcb:
                fcb = P1.tile([128, V], F32)
                nc.gpsimd.dma_start(fcb[:], fcb_d.ap().to_broadcast((128, V)))

            # persistent recurrence state / precompute outputs
            fpT = P1.tile([128, KT, R, BL], F16)       # feat_proj + attn_b
            xgxT = P1.tile([128, M3H, T * BL], F32)    # emb-side gate preacts
            h0 = P1.tile([128, KT, BL], F16)
            nc.vector.memset(h0[:], 0.0)
            h_lo = P1.tile([128, KT, 16 * BL], F16)    # h outputs, steps 0..15
            h_hi = P1.tile([128, KT, 16 * BL], F16)    # h outputs, steps 16..31

            # ---- precompute: feat_proj and xgx ----
            with tc.tile_pool(name="pre_ps", bufs=2, space="PSUM") as PPS:
                for mo in range(KT):
                    ps = PPS.tile([128, R * BL], F32, name="fp_ps")
                    for kt in range(KT):
                        nc.tensor.matmul(
                            ps[:], attn_We[:, kt, mo * 128:(mo + 1) * 128],
                            featsT[:, kt].rearrange("p r b -> p (r b)"),
                            start=(kt == 0), stop=(kt == KT - 1))
                    nc.vector.tensor_scalar(
                        out=fpT[:, mo].rearrange("p r b -> p (r b)"),
                        in0=ps[:], scalar1=attnb[:, mo], scalar2=None,
                        op0=OP.add)
                for m in range(M3H):
                    ps = PPS.tile([128, T * BL], F32, name="xg_ps")
                    for kt in range(KT):
                        nc.tensor.matmul(
                            ps[:], W_iheT[:, kt, m * 128:(m + 1) * 128],
                            embT[:, kt], start=(kt == 0), stop=(kt == KT - 1))
                    nc.scalar.add(xgxT[:, m], ps[:], add=bsum[:, m])

            # ---- recurrence + interleaved first-half fc ----
            with tc.tile_pool(name="scratch", bufs=2) as PSC, \
                 tc.tile_pool(name="gates", bufs=2) as PG, \
                 tc.tile_pool(name="ps_hp", bufs=1, space="PSUM") as PS_HP, \
                 tc.tile_pool(name="ps_sc", bufs=1, space="PSUM") as PS_SC, \
                 tc.tile_pool(name="ps_g", bufs=1, space="PSUM") as PS_G, \
                 tc.tile_pool(name="fc_ps", bufs=4, space="PSUM") as FPS, \
                 tc.tile_pool(name="fc_sb", bufs=3) as FSB:

                def fc_chunk(mo, ch, qsel, copy_eng="act"):
                    h_src = h_lo if mo == 0 else h_hi
                    rows = slice(mo * 128, (mo + 1) * 128)
                    nv = min(512, V - ch * 512)
                    cols = slice(ch * 512, ch * 512 + nv)
                    ps = FPS.tile([128, 512], F32, name="fc_ps")
                    for kt in range(KT):
                        nc.tensor.matmul(
                            ps[:, :nv], h_src[:, kt], fcW[:, kt, cols],
                            start=(kt == 0), stop=(kt == KT - 1))
                    ot = FSB.tile([128, 512], F16, name="fc_ot")
                    if has_fcb:
                        nc.vector.tensor_tensor(
                            out=ot[:, :nv], in0=ps[:, :nv], in1=fcb[:, cols],
                            op=OP.add)
                    elif copy_eng == "act":
                        nc.scalar.copy(ot[:, :nv], ps[:, :nv])
                    else:
                        nc.vector.tensor_copy(ot[:, :nv], ps[:, :nv])
                    nc.sync.dma_start(out_d.ap()[rows, cols], ot[:, :nv])

                # fc chunk schedule: first-half chunks spread over steps 16..31
                fc_sched = {}
                for s in range(16, T):
                    lo = (s - 16) * NCH // 16
                    hi = (s - 15) * NCH // 16
                    fc_sched[s] = list(range(lo, hi))

                for t in range(T):
                    if t == 0:
                        h_prev = h0[:]
                    elif t <= 16:
                        h_prev = h_lo[:, :, (t - 1) * BL:t * BL]
                    else:
                        h_prev = h_hi[:, :, (t - 17) * BL:(t - 16) * BL]
                    h_slab = h_lo if t < 16 else h_hi
                    hcol = (t % 16) * BL
                    xg = xgxT[:, :, t * BL:(t + 1) * BL]

                    # --- PE: h_proj first (it heads the energy chain) ---
                    hp = PS_HP.tile([128, KT, BL], F32, name="hp")
                    for mo in range(KT):
                        for kt in range(KT):
                            nc.tensor.matmul(
                                hp[:, mo], attn_Wh[:, kt, mo * 128:(mo + 1) * 128],
                                h_prev[:, kt], start=(kt == 0),
                                stop=(kt == KT - 1))
                    # gh fills PE while the energy chain runs
                    g_gh = PS_G.tile([128, M3H, BL], F32, name="g_gh")
                    g_cgx = PS_G.tile([128, M3H, BL], F32, name="g_cgx")
                    for m in range(M3H):
                        for kt in range(KT):
                            nc.tensor.matmul(
                                g_gh[:, m], W_hhT[:, kt, m * 128:(m + 1) * 128],
                                h_prev[:, kt], start=(kt == 0),
                                stop=(kt == KT - 1))

                    # --- energy: tanh(fp + hp) then scores, in two r-halves
                    hp16 = PSC.tile([128, KT, BL], F16, name="hp16")
                    nc.vector.tensor_copy(hp16[:], hp[:])
                    # scores land replicated on all 128 partitions (vw is
                    # column-replicated), so softmax needs no broadcast
                    sc = PS_SC.tile([128, R, BL], F32, name="sc", bufs=1)
                    en_b = PSC.tile([128, KT, R, BL], F16, name="en_b", bufs=1)
                    for (r0, r1) in RSPLIT:
                        nr = r1 - r0
                        en_f = PSC.tile([128, KT, 25, BL], F16,
                                        name=f"en_f{r0}", bufs=1)
                        nc.vector.tensor_tensor(
                            out=en_f[:, :, :nr], in0=fpT[:, :, r0:r1],
                            in1=hp16[:, :, None, :].to_broadcast(
                                (128, KT, nr, BL)),
                            op=OP.add)
                        nc.scalar.activation(
                            en_b[:, :, r0:r1], en_f[:, :, :nr], AF.Tanh)
                        for kt in range(KT):
                            nc.tensor.matmul(
                                sc[:, r0:r1].rearrange("p r b -> p (r b)"),
                                vw[:, kt],
                                en_b[:, kt, r0:r1].rearrange("p r b -> p (r b)"),
                                start=(kt == 0), stop=(kt == KT - 1))

                    # gate pre-adds that only need gh + xg: emitted after the
                    # energy chain so DVE prioritizes the chain
                    rzpre = PG.tile([128, 8, BL], F32, name="rzpre")
                    nc.vector.tensor_tensor(
                        out=rzpre[:], in0=g_gh[:, 0:8], in1=xg[:, 0:8],
                        op=OP.add)
                    # npre2 = xg_n + 0.5*gh_n  (n = tanh(npre2 + cgx_n
                    #                            + 0.5*t_r*gh_n - 0.5*gh_n
                    #   ... with r = 0.5 + 0.5*t_r: r*hn = 0.5*hn + 0.5*t_r*hn)
                    npre2 = PG.tile([128, 4, BL], F32, name="npre2")
                    nc.vector.scalar_tensor_tensor(
                        out=npre2[:], in0=g_gh[:, 8:12], scalar=0.5,
                        in1=xg[:, 8:12], op0=OP.mult, op1=OP.add)

                    # softmax (unnormalized; scores are O(1), no max-sub):
                    # exp reads replicated scores from PSUM, writes replicated
                    # fp16 weights straight to SBUF
                    exb = PSC.tile([128, BL, R], F16, name="exb", bufs=1)
                    nc.scalar.activation(
                        exb[:].rearrange("p b r -> p r b"), sc[:], AF.Exp)
                    # s_sum = sum_r exb as a TT tree on the idle GpSimd engine
                    # (keeps the softmax denominator off the DVE chain)
                    st = PSC.tile([128, BL, 25], F32, name="st")
                    nc.gpsimd.tensor_tensor(
                        out=st[:, :, 0:24], in0=exb[:, :, 0:24],
                        in1=exb[:, :, 25:49], op=OP.add)
                    nc.gpsimd.tensor_tensor(
                        out=st[:, :, 0:12], in0=st[:, :, 0:12],
                        in1=st[:, :, 12:24], op=OP.add)
                    nc.gpsimd.tensor_tensor(
                        out=st[:, :, 0:6], in0=st[:, :, 0:6],
                        in1=st[:, :, 6:12], op=OP.add)
                    nc.gpsimd.tensor_tensor(
                        out=st[:, :, 0:3], in0=st[:, :, 0:3],
                        in1=st[:, :, 3:6], op=OP.add)
                    nc.gpsimd.tensor_tensor(
                        out=st[:, :, 0:1], in0=st[:, :, 0:1],
                        in1=st[:, :, 1:2], op=OP.add)
                    nc.gpsimd.tensor_tensor(
                        out=st[:, :, 0:1], in0=st[:, :, 0:1],
                        in1=st[:, :, 2:3], op=OP.add)
                    nc.gpsimd.tensor_tensor(
                        out=st[:, :, 0:1], in0=st[:, :, 0:1],
                        in1=exb[:, :, 24:25], op=OP.add)
                    # context = (sum_r exb * feats) * recb; the r-reduction is
                    # two-stage: fold r-halves with a 2x fp16 add, then reduce
                    prod = PSC.tile([128, KT, BL, R], F16, name="prod", bufs=1)
                    nc.vector.tensor_tensor(
                        out=prod[:], in0=feats16[:],
                        in1=exb[:, None].to_broadcast((128, KT, BL, R)),
                        op=OP.mult)
                    fold = PSC.tile([128, KT, BL, 25], F16, name="fold")
                    nc.vector.tensor_tensor(
                        out=fold[:, :, :, 0:24], in0=prod[:, :, :, 0:24],
                        in1=prod[:, :, :, 25:49], op=OP.add)
                    nc.vector.tensor_copy(fold[:, :, :, 24:25],
                                          prod[:, :, :, 24:25])
                    ctx_u = PSC.tile([128, KT, BL], F32, name="ctx_u")
                    nc.vector.tensor_reduce(
                        out=ctx_u[:], in_=fold[:],
                        axis=mybir.AxisListType.X, op=OP.add)
                    recb = PSC.tile([128, BL], F32, name="recb")
                    nc.vector.reciprocal(recb[:], st[:, :, 0])
                    ctx16 = PSC.tile([128, KT, BL], F16, name="ctx16")
                    nc.vector.tensor_tensor(
                        out=ctx16[:], in0=ctx_u[:],
                        in1=recb[:, None, :].to_broadcast((128, KT, BL)),
                        op=OP.mult)

                    # cgx = W_ih[:, E:] @ context
                    for m in range(M3H):
                        for kt in range(KT):
                            nc.tensor.matmul(
                                g_cgx[:, m], W_ihcT[:, kt, m * 128:(m + 1) * 128],
                                ctx16[:, kt], start=(kt == 0),
                                stop=(kt == KT - 1))

                    # --- gates (sigmoid = 0.5 + 0.5*tanh(x/2), kept as raw
                    # t = tanh(x/2) and folded algebraically) ---
                    rz_t = PG.tile([128, 8, BL], F16, name="rz_t")
                    nc.vector.tensor_tensor(
                        out=rz_t[:], in0=g_cgx[:, 0:8], in1=rzpre[:],
                        op=OP.add)
                    t_rz = PG.tile([128, 8, BL], F16, name="t_rz")
                    nc.scalar.activation(t_rz[:], rz_t[:], AF.Tanh, scale=0.5)
                    # tmp_n = npre2 + cgx_n  (during the t_rz activation)
                    tmp_n = PG.tile([128, 4, BL], F32, name="tmp_n")
                    nc.vector.tensor_tensor(
                        out=tmp_n[:], in0=npre2[:], in1=g_cgx[:, 8:12],
                        op=OP.add)
                    # n = tanh(tmp_n + 0.5*t_r*gh_n)
                    q = PG.tile([128, 4, BL], F32, name="q")
                    nc.vector.tensor_tensor(
                        out=q[:], in0=t_rz[:, 0:4], in1=g_gh[:, 8:12],
                        op=OP.mult)
                    n_p = PG.tile([128, 4, BL], F16, name="n_p")
                    nc.vector.scalar_tensor_tensor(
                        out=n_p[:], in0=q[:], scalar=0.5, in1=tmp_n[:],
                        op0=OP.mult, op1=OP.add)
                    n_t = PG.tile([128, 4, BL], F16, name="n_t")
                    nc.scalar.activation(n_t[:], n_p[:], AF.Tanh)
                    # h_new = n + z*(h-n), z = 0.5 + 0.5*t_z:
                    #   d = h - n; u = (t_z + 1)*d; h_new = 0.5*u + n
                    d = PG.tile([128, 4, BL], F16, name="d")
                    nc.vector.tensor_tensor(
                        out=d[:], in0=h_prev, in1=n_t[:], op=OP.subtract)
                    u = PG.tile([128, 4, BL], F16, name="u")
                    nc.vector.scalar_tensor_tensor(
                        out=u[:], in0=t_rz[:, 4:8], scalar=1.0, in1=d[:],
                        op0=OP.add, op1=OP.mult)
                    nc.vector.scalar_tensor_tensor(
                        out=h_slab[:, :, hcol:hcol + BL], in0=u[:], scalar=0.5,
                        in1=n_t[:], op0=OP.mult, op1=OP.add)

                    # interleave first-half fc chunks on the idle PE
                    for ch in fc_sched.get(t, ()):
                        fc_chunk(0, ch, ch)

                # ---- tail: second-half fc (copies alternate ACT/DVE so the
                # PE streams without waiting on the PSUM pool) ----
                for ch in range(NCH):
                    fc_chunk(1, ch, ch, copy_eng="act" if ch % 2 == 0 else "dve")

    nc.compile()
    return nc


def _get_built(has_fcb=True):
    with _BUILD_LOCK:
        if has_fcb not in _BUILT:
            _BUILT[has_fcb] = _build(has_fcb)
    return _BUILT[has_fcb]


def kernel(features, captions, embed_table, attn_W, attn_b, v_w,
           W_ih, W_hh, b_ih, b_hh, fc_W, fc_b):
    from concourse.bass_utils import run_bass_kernel_spmd

    features = np.asarray(features, dtype=np.float32)
    captions = np.asarray(captions)
    embed_table = np.asarray(embed_table, dtype=np.float32)
    attn_W = np.asarray(attn_W, dtype=np.float32)
    attn_b = np.asarray(attn_b, dtype=np.float32)
    v_w = np.asarray(v_w, dtype=np.float32)
    W_ih = np.asarray(W_ih, dtype=np.float32)
    W_hh = np.asarray(W_hh, dtype=np.float32)
    b_ih = np.asarray(b_ih, dtype=np.float32)
    b_hh = np.asarray(b_hh, dtype=np.float32)
    fc_W = np.asarray(fc_W, dtype=np.float32)
    fc_b = np.asarray(fc_b, dtype=np.float32)

    has_fcb = bool(np.any(fc_b))
    nc = _get_built(has_fcb)

    f16 = np.float16
    shared = {
        "attn_We": attn_W[:E].astype(f16),
        "attn_Wh": attn_W[E:].astype(f16),
        "W_hhT": np.ascontiguousarray(W_hh.T).astype(f16),
        "W_ihcT": np.ascontiguousarray(W_ih[:, E:].T).astype(f16),
        "W_iheT": np.ascontiguousarray(W_ih[:, :E].T).astype(f16),
        "vw": np.repeat(v_w[:, None], 128, axis=1).astype(f16),
        "bsum": np.ascontiguousarray((b_ih + b_hh)[:, None]),
        "attnb": np.ascontiguousarray(attn_b[:, None]),
        "fcW": fc_W.astype(f16),
    }
    if has_fcb:
        shared["fcb"] = np.ascontiguousarray(fc_b[None, :])
    emb = embed_table[captions[:, :T].astype(np.int64)]  # [B, T, E]
    in_maps = []
    for c in range(NCORES):
        rows = slice(c * BL, (c + 1) * BL)
        m = dict(shared)
        m["featsT"] = features[rows].transpose(2, 1, 0).astype(f16)
        m["featsb"] = features[rows].transpose(2, 0, 1).astype(f16)
        m["embT"] = emb[rows].transpose(2, 1, 0).reshape(E, T * BL).astype(f16)
        in_maps.append(m)

    res = run_bass_kernel_spmd(nc, in_maps, core_ids=list(range(NCORES)))

    out = np.empty((B, T, V), dtype=np.float32)
    for c in range(NCORES):
        # rows of per-core output are t*BL + b_local
        out[c * BL:(c + 1) * BL] = (
            res.results[c]["out"].astype(np.float32)
            .reshape(T, BL, V).transpose(1, 0, 2))
    return out


# revision 25
# speedup vs baseline: 1.0780x; 1.0131x over previous
"""Trainium2 Bass kernel for nn_DecoderGRU (attention GRU decoder + vocab head).

Strategy (8 NeuronCores, data-parallel over batch, 8 rows/core):
  - All-fp16 tensors (weights, activations, history): halves DMA traffic and
    doubles DVE throughput where 2x modes apply; PSUM accumulation is fp32.
  - Startup: precompute inputs (featsT/attn_We/embT/W_iheT) DMA'd first,
    recurrence weights next, fcW last so the recurrence starts ~20us in.
  - Hoisted out of the 32-step loop: feat_proj, xgx (= emb @ W_ih[:, :E].T
    + b_ih + b_hh), and the fc head.
  - fc head: stationary = 128 finished h columns; the first half (steps
    0..15) is interleaved into steps 16..31 on the otherwise idle PE, with
    logits DMA'd straight from PSUM to DRAM; only the second half runs as a
    tail after the loop.
  - Per step, the serial chain is minimized: hp before gh on PE; gate
    pre-adds emitted after the energy chain; exb/recb PSUM->SBUF copies on
    ACT (frees DVE); sigmoid via raw tanh(x/2) algebra with
    scalar_tensor_tensor fusions; h_new written directly into the fp16
    history slab used by both the next step and the fc head.
"""

import threading

import numpy as np

B, R, E, H, V, L = 64, 49, 512, 512, 10000, 33
T = L - 1            # 32 decode steps
NCORES = 8
BL = B // NCORES     # 8 batch rows per core
KT = E // 128        # 4 k-tiles of 128 for E=H=512
M3H = (3 * H) // 128  # 12 m-tiles for gate dim
RSPLIT = ((0, 25), (25, 49))  # r-halves for the energy pipeline
NCH = (V + 511) // 512        # 20 vocab chunks, last = 272

_BUILD_LOCK = threading.Lock()
_BUILT = {}


def _build(has_fcb=True):
    import concourse.mybir as mybir
    import concourse.tile as tile
    from concourse import bacc

    F32 = mybir.dt.float32
    F16 = mybir.dt.float16
    AF = mybir.ActivationFunctionType
    OP = mybir.AluOpType

    nc = bacc.Bacc("TRN2", target_bir_lowering=False, debug=False,
                   num_devices=NCORES)

    # ---- DRAM I/O (everything fp16 except f32 biases and the output) ----
    featsT_d = nc.dram_tensor("featsT", [E, R, BL], F16, kind="ExternalInput")
    featsb_d = nc.dram_tensor("featsb", [E, BL, R], F16, kind="ExternalInput")
    embT_d = nc.dram_tensor("embT", [E, T * BL], F16, kind="ExternalInput")
    attn_We_d = nc.dram_tensor("attn_We", [E, H], F16, kind="ExternalInput")
    attn_Wh_d = nc.dram_tensor("attn_Wh", [H, H], F16, kind="ExternalInput")
    W_hhT_d = nc.dram_tensor("W_hhT", [H, 3 * H], F16, kind="ExternalInput")
    W_ihcT_d = nc.dram_tensor("W_ihcT", [E, 3 * H], F16, kind="ExternalInput")
    W_iheT_d = nc.dram_tensor("W_iheT", [E, 3 * H], F16, kind="ExternalInput")
    vw_d = nc.dram_tensor("vw", [H, 1], F16, kind="ExternalInput")
    bsum_d = nc.dram_tensor("bsum", [3 * H, 1], F32, kind="ExternalInput")
    attnb_d = nc.dram_tensor("attnb", [H, 1], F32, kind="ExternalInput")
    fcW_d = nc.dram_tensor("fcW", [H, V], F16, kind="ExternalInput")
    out_d = nc.dram_tensor("out", [T * BL, V], F16, kind="ExternalOutput")
    if has_fcb:
        fcb_d = nc.dram_tensor("fcb", [1, V], F32, kind="ExternalInput")

    r3 = lambda ap: ap.rearrange("(kt p) m -> p kt m", p=128)

    with tile.TileContext(nc) as tc:
        with tc.tile_pool(name="persist", bufs=1) as P1:
            # ---- input DMAs, ordered so the DMA device serves the
            # precompute first, recurrence weights next, fcW last ----
            featsT = P1.tile([128, KT, R, BL], F16)
            nc.sync.dma_start(featsT[:], featsT_d.ap().rearrange(
                "(kt p) r b -> p kt r b", p=128))
            attn_We = P1.tile([128, KT, H], F16)
            nc.sync.dma_start(attn_We[:], r3(attn_We_d.ap()))
            embT = P1.tile([128, KT, T * BL], F16)
            nc.scalar.dma_start(embT[:], r3(embT_d.ap()))
            W_iheT = P1.tile([128, KT, 3 * H], F16)
            nc.scalar.dma_start(W_iheT[:], r3(W_iheT_d.ap()))

            attnb = P1.tile([128, KT, 1], F32)
            nc.gpsimd.dma_start(attnb[:], r3(attnb_d.ap()))
            bsum = P1.tile([128, M3H, 1], F32)
            nc.gpsimd.dma_start(bsum[:], r3(bsum_d.ap()))
            vw1 = P1.tile([128, KT, 1], F16)
            nc.gpsimd.dma_start(vw1[:], r3(vw_d.ap()))
            attn_Wh = P1.tile([128, KT, H], F16)
            nc.gpsimd.dma_start(attn_Wh[:], r3(attn_Wh_d.ap()))
            W_hhT = P1.tile([128, KT, 3 * H], F16)
            nc.gpsimd.dma_start(W_hhT[:], r3(W_hhT_d.ap()))
            W_ihcT = P1.tile([128, KT, 3 * H], F16)
            nc.gpsimd.dma_start(W_ihcT[:], r3(W_ihcT_d.ap()))
            fcW = P1.tile([128, KT, V], F16)
            for kt in range(KT):
                nc.gpsimd.dma_start(fcW[:, kt], r3(fcW_d.ap())[:, kt])
            if has_fcb:
                fcb = P1.tile([128, V], F32)
                nc.gpsimd.dma_start(fcb[:], fcb_d.ap().to_broadcast((128, V)))

            # derived on-device from featsT / vw1 (cheaper than extra DMAs)
            feats16 = P1.tile([128, KT, BL, R], F16)
            nc.vector.tensor_copy(
                feats16[:], featsT[:].rearrange("p kt r b -> p kt b r"))
            vw = P1.tile([128, KT, 128], F16)
            nc.vector.tensor_copy(
                vw[:], vw1[:, :, :].to_broadcast((128, KT, 128)))

            # persistent recurrence state / precompute outputs
            fpT = P1.tile([128, KT, R, BL], F16)       # feat_proj + attn_b
            xgxT = P1.tile([128, M3H, T * BL], F32)    # emb-side gate preacts
            h0 = P1.tile([128, KT, BL], F16)
            nc.vector.memset(h0[:], 0.0)
            h_lo = P1.tile([128, KT, 16 * BL], F16)    # h outputs, steps 0..15
            h_hi = P1.tile([128, KT, 16 * BL], F16)    # h outputs, steps 16..31

            # ---- precompute: feat_proj and xgx ----
            with tc.tile_pool(name="pre_ps", bufs=2, space="PSUM") as PPS:
                for mo in range(KT):
                    ps = PPS.tile([128, R * BL], F32, name="fp_ps")
                    for kt in range(KT):
                        nc.tensor.matmul(
                            ps[:], attn_We[:, kt, mo * 128:(mo + 1) * 128],
                            featsT[:, kt].rearrange("p r b -> p (r b)"),
                            start=(kt == 0), stop=(kt == KT - 1))
                    nc.vector.tensor_scalar(
                        out=fpT[:, mo].rearrange("p r b -> p (r b)"),
                        in0=ps[:], scalar1=attnb[:, mo], scalar2=None,
                        op0=OP.add)
                for m in range(M3H):
                    ps = PPS.tile([128, T * BL], F32, name="xg_ps")
                    for kt in range(KT):
                        nc.tensor.matmul(
                            ps[:], W_iheT[:, kt, m * 128:(m + 1) * 128],
                            embT[:, kt], start=(kt == 0), stop=(kt == KT - 1))
                    if m % 2 == 0:
                        nc.scalar.add(xgxT[:, m], ps[:], add=bsum[:, m])
                    else:
                        nc.vector.tensor_scalar(
                            out=xgxT[:, m], in0=ps[:], scalar1=bsum[:, m],
                            scalar2=None, op0=OP.add)

            # ---- recurrence + interleaved first-half fc ----
            with tc.tile_pool(name="scratch", bufs=2) as PSC, \
                 tc.tile_pool(name="gates", bufs=2) as PG, \
                 tc.tile_pool(name="ps_hp", bufs=1, space="PSUM") as PS_HP, \
                 tc.tile_pool(name="ps_sc", bufs=1, space="PSUM") as PS_SC, \
                 tc.tile_pool(name="ps_g", bufs=1, space="PSUM") as PS_G, \
                 tc.tile_pool(name="fc_ps", bufs=4, space="PSUM") as FPS, \
                 tc.tile_pool(name="fc_sb", bufs=8) as FSB:

                def fc_chunk(mo, ch, qsel, copy_eng="act"):
                    h_src = h_lo if mo == 0 else h_hi
                    rows = slice(mo * 128, (mo + 1) * 128)
                    nv = min(512, V - ch * 512)
                    cols = slice(ch * 512, ch * 512 + nv)
                    ps = FPS.tile([128, 512], F32, name="fc_ps")
                    for kt in range(KT):
                        nc.tensor.matmul(
                            ps[:, :nv], h_src[:, kt], fcW[:, kt, cols],
                            start=(kt == 0), stop=(kt == KT - 1))
                    ot = FSB.tile([128, 512], F16, name="fc_ot")
                    if has_fcb:
                        nc.vector.tensor_tensor(
                            out=ot[:, :nv], in0=ps[:, :nv], in1=fcb[:, cols],
                            op=OP.add)
                    elif copy_eng == "act":
                        nc.scalar.copy(ot[:, :nv], ps[:, :nv])
                    else:
                        nc.vector.tensor_copy(ot[:, :nv], ps[:, :nv])
                    nc.sync.dma_start(out_d.ap()[rows, cols], ot[:, :nv])

                # fc chunk schedule: first-half chunks spread over steps 16..31
                fc_sched = {}
                for s in range(16, T):
                    lo = (s - 16) * NCH // 16
                    hi = (s - 15) * NCH // 16
                    fc_sched[s] = list(range(lo, hi))

                for t in range(T):
                    if t == 0:
                        h_prev = h0[:]
                    elif t <= 16:
                        h_prev = h_lo[:, :, (t - 1) * BL:t * BL]
                    else:
                        h_prev = h_hi[:, :, (t - 17) * BL:(t - 16) * BL]
                    h_slab = h_lo if t < 16 else h_hi
                    hcol = (t % 16) * BL
                    xg = xgxT[:, :, t * BL:(t + 1) * BL]

                    # --- PE: h_proj first (it heads the energy chain) ---
                    hp = PS_HP.tile([128, KT, BL], F32, name="hp")
                    for mo in range(KT):
                        for kt in range(KT):
                            nc.tensor.matmul(
                                hp[:, mo], attn_Wh[:, kt, mo * 128:(mo + 1) * 128],
                                h_prev[:, kt], start=(kt == 0),
                                stop=(kt == KT - 1))
                    # gh fills PE while the energy chain runs
                    g_gh = PS_G.tile([128, M3H, BL], F32, name="g_gh")
                    g_cgx = PS_G.tile([128, M3H, BL], F32, name="g_cgx")
                    for m in range(M3H):
                        for kt in range(KT):
                            nc.tensor.matmul(
                                g_gh[:, m], W_hhT[:, kt, m * 128:(m + 1) * 128],
                                h_prev[:, kt], start=(kt == 0),
                                stop=(kt == KT - 1))

                    # --- energy: tanh(fp + hp) then scores, in two r-halves
                    hp16 = PSC.tile([128, KT, BL], F16, name="hp16")
                    nc.vector.tensor_copy(hp16[:], hp[:])
                    # scores land replicated on all 128 partitions (vw is
                    # column-replicated), so softmax needs no broadcast
                    sc = PS_SC.tile([128, R, BL], F32, name="sc", bufs=1)
                    en_b = PSC.tile([128, KT, R, BL], F16, name="en_b", bufs=1)
                    for (r0, r1) in RSPLIT:
                        nr = r1 - r0
                        en_f = PSC.tile([128, KT, 25, BL], F16,
                                        name=f"en_f{r0}", bufs=1)
                        nc.vector.tensor_tensor(
                            out=en_f[:, :, :nr], in0=fpT[:, :, r0:r1],
                            in1=hp16[:, :, None, :].to_broadcast(
                                (128, KT, nr, BL)),
                            op=OP.add)
                        nc.scalar.activation(
                            en_b[:, :, r0:r1], en_f[:, :, :nr], AF.Tanh)
                        for kt in range(KT):
                            nc.tensor.matmul(
                                sc[:, r0:r1].rearrange("p r b -> p (r b)"),
                                vw[:, kt],
                                en_b[:, kt, r0:r1].rearrange("p r b -> p (r b)"),
                                start=(kt == 0), stop=(kt == KT - 1))

                    # gate pre-adds that only need gh + xg: emitted after the
                    # energy chain so DVE prioritizes the chain
                    rzpre = PG.tile([128, 8, BL], F32, name="rzpre")
                    nc.vector.tensor_tensor(
                        out=rzpre[:], in0=g_gh[:, 0:8], in1=xg[:, 0:8],
                        op=OP.add)
                    # npre2 = xg_n + 0.5*gh_n  (n = tanh(npre2 + cgx_n
                    #                            + 0.5*t_r*gh_n - 0.5*gh_n
                    #   ... with r = 0.5 + 0.5*t_r: r*hn = 0.5*hn + 0.5*t_r*hn)
                    npre2 = PG.tile([128, 4, BL], F32, name="npre2")
                    nc.vector.scalar_tensor_tensor(
                        out=npre2[:], in0=g_gh[:, 8:12], scalar=0.5,
                        in1=xg[:, 8:12], op0=OP.mult, op1=OP.add)

                    # softmax (unnormalized; scores are O(1), no max-sub):
                    # exp reads replicated scores from PSUM, writes replicated
                    # fp16 weights straight to SBUF
                    exb = PSC.tile([128, BL, R], F16, name="exb", bufs=1)
                    nc.scalar.activation(
                        exb[:].rearrange("p b r -> p r b"), sc[:], AF.Exp)
                    # s_sum = sum_r exb as a TT tree on the idle GpSimd engine
                    # (keeps the softmax denominator off the DVE chain)
                    st = PSC.tile([128, BL, 25], F32, name="st")
                    nc.gpsimd.tensor_tensor(
                        out=st[:, :, 0:24], in0=exb[:, :, 0:24],
                        in1=exb[:, :, 25:49], op=OP.add)
                    nc.gpsimd.tensor_tensor(
                        out=st[:, :, 0:12], in0=st[:, :, 0:12],
                        in1=st[:, :, 12:24], op=OP.add)
                    nc.gpsimd.tensor_tensor(
                        out=st[:, :, 0:6], in0=st[:, :, 0:6],
                        in1=st[:, :, 6:12], op=OP.add)
                    nc.gpsimd.tensor_tensor(
                        out=st[:, :, 0:3], in0=st[:, :, 0:3],
                        in1=st[:, :, 3:6], op=OP.add)
                    nc.gpsimd.tensor_tensor(
                        out=st[:, :, 0:1], in0=st[:, :, 0:1],
                        in1=st[:, :, 1:2], op=OP.add)
                    nc.gpsimd.tensor_tensor(
                        out=st[:, :, 0:1], in0=st[:, :, 0:1],
                        in1=st[:, :, 2:3], op=OP.add)
                    nc.gpsimd.tensor_tensor(
                        out=st[:, :, 0:1], in0=st[:, :, 0:1],
                        in1=exb[:, :, 24:25], op=OP.add)
                    # context = (sum_r exb * feats) * recb; the r-reduction is
                    # two-stage: fold r-halves with a 2x fp16 add, then reduce
                    prod = PSC.tile([128, KT, BL, R], F16, name="prod", bufs=1)
                    nc.vector.tensor_tensor(
                        out=prod[:], in0=feats16[:],
                        in1=exb[:, None].to_broadcast((128, KT, BL, R)),
                        op=OP.mult)
                    fold = PSC.tile([128, KT, BL, 25], F16, name="fold")
                    nc.vector.tensor_tensor(
                        out=fold[:, :, :, 0:24], in0=prod[:, :, :, 0:24],
                        in1=prod[:, :, :, 25:49], op=OP.add)
                    nc.vector.tensor_copy(fold[:, :, :, 24:25],
                                          prod[:, :, :, 24:25])
                    ctx_u = PSC.tile([128, KT, BL], F32, name="ctx_u")
                    nc.vector.tensor_reduce(
                        out=ctx_u[:], in_=fold[:],
                        axis=mybir.AxisListType.X, op=OP.add)
                    recb = PSC.tile([128, BL], F32, name="recb")
                    nc.vector.reciprocal(recb[:], st[:, :, 0])
                    ctx16 = PSC.tile([128, KT, BL], F16, name="ctx16")
                    nc.vector.tensor_tensor(
                        out=ctx16[:], in0=ctx_u[:],
                        in1=recb[:, None, :].to_broadcast((128, KT, BL)),
                        op=OP.mult)

                    # cgx = W_ih[:, E:] @ context
                    for m in range(M3H):
                        for kt in range(KT):
                            nc.tensor.matmul(
                                g_cgx[:, m], W_ihcT[:, kt, m * 128:(m + 1) * 128],
                                ctx16[:, kt], start=(kt == 0),
                                stop=(kt == KT - 1))

                    # --- gates (sigmoid = 0.5 + 0.5*tanh(x/2), kept as raw
                    # t = tanh(x/2) and folded algebraically) ---
                    rz_t = PG.tile([128, 8, BL], F16, name="rz_t")
                    nc.vector.tensor_tensor(
                        out=rz_t[:], in0=g_cgx[:, 0:8], in1=rzpre[:],
                        op=OP.add)
                    t_rz = PG.tile([128, 8, BL], F16, name="t_rz")
                    nc.scalar.activation(t_rz[:], rz_t[:], AF.Tanh, scale=0.5)
                    # tmp_n = npre2 + cgx_n  (during the t_rz activation)
                    tmp_n = PG.tile([128, 4, BL], F32, name="tmp_n")
                    nc.vector.tensor_tensor(
                        out=tmp_n[:], in0=npre2[:], in1=g_cgx[:, 8:12],
                        op=OP.add)
                    # n = tanh(tmp_n + 0.5*t_r*gh_n)
                    q = PG.tile([128, 4, BL], F32, name="q")
                    nc.vector.tensor_tensor(
                        out=q[:], in0=t_rz[:, 0:4], in1=g_gh[:, 8:12],
                        op=OP.mult)
                    n_p = PG.tile([128, 4, BL], F16, name="n_p")
                    nc.vector.scalar_tensor_tensor(
                        out=n_p[:], in0=q[:], scalar=0.5, in1=tmp_n[:],
                        op0=OP.mult, op1=OP.add)
                    n_t = PG.tile([128, 4, BL], F16, name="n_t")
                    nc.scalar.activation(n_t[:], n_p[:], AF.Tanh)
                    # h_new = n + z*(h-n), z = 0.5 + 0.5*t_z:
                    #   d = h - n; u = (t_z + 1)*d; h_new = 0.5*u + n
                    d = PG.tile([128, 4, BL], F16, name="d")
                    nc.vector.tensor_tensor(
                        out=d[:], in0=h_prev, in1=n_t[:], op=OP.subtract)
                    u = PG.tile([128, 4, BL], F16, name="u")
                    nc.vector.scalar_tensor_tensor(
                        out=u[:], in0=t_rz[:, 4:8], scalar=1.0, in1=d[:],
                        op0=OP.add, op1=OP.mult)
                    nc.vector.scalar_tensor_tensor(
                        out=h_slab[:, :, hcol:hcol + BL], in0=u[:], scalar=0.5,
                        in1=n_t[:], op0=OP.mult, op1=OP.add)

                    # interleave first-half fc chunks on the idle PE
                    for ch in fc_sched.get(t, ()):
                        fc_chunk(0, ch, ch)

                # ---- tail: second-half fc (copies alternate ACT/DVE so the
                # PE streams without waiting on the PSUM pool) ----
                for ch in range(NCH):
                    fc_chunk(1, ch, ch, copy_eng="act" if ch % 2 == 0 else "dve")

    nc.compile()
    return nc


def _get_built(has_fcb=True):
    with _BUILD_LOCK:
        if has_fcb not in _BUILT:
            _BUILT[has_fcb] = _build(has_fcb)
    return _BUILT[has_fcb]


def kernel(features, captions, embed_table, attn_W, attn_b, v_w,
           W_ih, W_hh, b_ih, b_hh, fc_W, fc_b):
    from concourse.bass_utils import run_bass_kernel_spmd

    features = np.asarray(features, dtype=np.float32)
    captions = np.asarray(captions)
    embed_table = np.asarray(embed_table, dtype=np.float32)
    attn_W = np.asarray(attn_W, dtype=np.float32)
    attn_b = np.asarray(attn_b, dtype=np.float32)
    v_w = np.asarray(v_w, dtype=np.float32)
    W_ih = np.asarray(W_ih, dtype=np.float32)
    W_hh = np.asarray(W_hh, dtype=np.float32)
    b_ih = np.asarray(b_ih, dtype=np.float32)
    b_hh = np.asarray(b_hh, dtype=np.float32)
    fc_W = np.asarray(fc_W, dtype=np.float32)
    fc_b = np.asarray(fc_b, dtype=np.float32)

    has_fcb = bool(np.any(fc_b))
    nc = _get_built(has_fcb)

    f16 = np.float16
    shared = {
        "attn_We": attn_W[:E].astype(f16),
        "attn_Wh": attn_W[E:].astype(f16),
        "W_hhT": np.ascontiguousarray(W_hh.T).astype(f16),
        "W_ihcT": np.ascontiguousarray(W_ih[:, E:].T).astype(f16),
        "W_iheT": np.ascontiguousarray(W_ih[:, :E].T).astype(f16),
        "vw": v_w[:, None].astype(f16),
        "bsum": np.ascontiguousarray((b_ih + b_hh)[:, None]),
        "attnb": np.ascontiguousarray(attn_b[:, None]),
        "fcW": fc_W.astype(f16),
    }
    if has_fcb:
        shared["fcb"] = np.ascontiguousarray(fc_b[None, :])
    emb = embed_table[captions[:, :T].astype(np.int64)]  # [B, T, E]
    in_maps = []
    for c in range(NCORES):
        rows = slice(c * BL, (c + 1) * BL)
        m = dict(shared)
        m["featsT"] = features[rows].transpose(2, 1, 0).astype(f16)
        m["featsb"] = features[rows].transpose(2, 0, 1).astype(f16)
        m["embT"] = emb[rows].transpose(2, 1, 0).reshape(E, T * BL).astype(f16)
        in_maps.append(m)

    res = run_bass_kernel_spmd(nc, in_maps, core_ids=list(range(NCORES)))

    out = np.empty((B, T, V), dtype=np.float32)
    for c in range(NCORES):
        # rows of per-core output are t*BL + b_local
        out[c * BL:(c + 1) * BL] = (
            res.results[c]["out"].astype(np.float32)
            .reshape(T, BL, V).transpose(1, 0, 2))
    return out
